# revision 3
# baseline (speedup 1.0000x reference)
"""DSSM (S4D-style) chunked state-space conv kernel for Trainium2, 8 cores.

Math: y[b,h,:] = causal_conv(u_masked[b,h,:], K[h,:]) + D[h]*u_masked, masked,
where K[h,l] = 2*Re(sum_n Cs[h,n] * w[h,n]^l), w = exp(dt*A).

Chunked algorithm (T=256 chunk, J=16 chunks, N=64 complex states):
  intra-chunk : Toeplitz matmul with K[0:256] (D folded into K[0])
  inter-chunk : A_j = V^T u_j, E_{j+1} = p*E_j + A_j (complex scan, DVE),
                y_state = W^T E_j
The cross-half Toeplitz block T2 is not transferred: T2^T u0 = W1^T(V_hat^T u0)
with P0 = V_hat^T u0 emitted by phase A (V_hat already on chip for phase A).
The scan writes states directly into the phase-C (h, jl, b) layout (no mirror
copies).  Phase C runs in [tau, pair] orientation: weights are the stationary
operand, ragged (jl, b) pairs stream; per jh-half only nb_half alive batches
are packed/transferred/computed (batches sorted by length desc).

Sharding: H=256 split across 8 cores (32 each).  Host does masking, sorting,
weight gen, layout packing, bf16 casts, and unshard+mask.
"""

import numpy as np
import ml_dtypes

import concourse.bass as bass
import concourse.bacc as bacc
import concourse.mybir as mybir
import concourse.tile as tile
from concourse.bass_utils import run_bass_kernel_spmd

H, N, B, L = 256, 64, 16, 4096
NCORES = 8
HC = H // NCORES            # 32 channels per core
T, J = 256, 16              # chunk length, number of chunks
JH = 2                      # halves of 8 chunks
N2 = 2 * N                  # 128 state rows (re, im)

F32 = mybir.dt.float32
BF16 = mybir.dt.bfloat16
NP_BF16 = ml_dtypes.bfloat16


def _plan(k_b):
    """Static schedule parameters from sorted per-batch chunk counts."""
    nb_scan = [sum(1 for k in k_b if k > j + 1) for j in range(J)]  # S_j users
    nb_half = [sum(1 for k in k_b if k > jh * 8) for jh in range(JH)]
    return nb_scan, nb_half


def _build_program(k_b):
    nb_scan, nb_half = _plan(k_b)
    nb0, nb1 = nb_half
    C0, C1 = 8 * nb0, 8 * nb1          # (jl, b) cols per (h, sb) per half
    R0, R1 = C0, C1                     # streamed pair cols in phase C
    HG = 8                              # h-groups of 4 for DMA/pipelining

    # ---- input DRAM layout (cols, in DMA order) ----
    # p chunk     : [ pre : HC | pim : HC ]   (compact; bcast on-device)
    # chunk g (x8): [ v_g : 4h x 256 | u0_g : 4h x 2sb x C0 ]
    # u1 chunk    : [ 32h x 2sb x C1 ]
    # w chunk g   : [ 4h x 384 ]  (T1 128 | W1 128 | W2 128)
    vg_c, u0g_c = 4 * 256, 4 * 2 * C0
    g_c = vg_c + u0g_c
    p_off = 0
    vu_off = 2 * HC
    u1_off = vu_off + 8 * g_c
    w_off = u1_off + HC * 2 * C1
    ICOLS = w_off + HC * 384

    nc = bacc.Bacc("TRN2", target_bir_lowering=False, debug=False,
                   enable_asserts=False, num_devices=NCORES)

    in_d = nc.dram_tensor("inp", [128, ICOLS], BF16, kind="ExternalInput")
    y0_d = nc.dram_tensor("y0", [128, HC * 2 * R0], BF16,
                          kind="ExternalOutput")
    y1_d = (nc.dram_tensor("y1", [128, HC * 2 * R1], BF16,
                           kind="ExternalOutput") if nb1 else None)

    with tile.TileContext(nc) as tc:
        with (
            tc.tile_pool(name="const", bufs=1) as cpool,
            tc.tile_pool(name="scantmp", bufs=2) as spool,
            tc.tile_pool(name="ysb", bufs=8) as ypool,
            tc.tile_pool(name="psum", bufs=8, space="PSUM") as psum,
        ):
            in_t = cpool.tile([128, ICOLS], BF16, name="in_t")
            a0 = cpool.tile([128, HC * C0], BF16, name="a0")   # (h, jl, b)
            a1 = (cpool.tile([128, HC * C1], BF16, name="a1") if nb1
                  else None)
            p0_0 = cpool.tile([128, HC * C0], BF16, name="p0_0")  # V_hat^T u0
            p0_1 = (cpool.tile([128, HC * C1], BF16, name="p0_1") if nb1
                    else None)
            pre_t = cpool.tile([128, HC * 16], BF16, name="pre_t")
            pim_t = cpool.tile([128, HC * 16], BF16, name="pim_t")
            # states E_c (entering chunk c), layout (h, jl, b) per half
            sy0 = cpool.tile([128, HC * R0], BF16, name="sy0")
            sy1 = (cpool.tile([128, HC * R1], BF16, name="sy1") if nb1
                   else None)

            # ---- DMAs (SP engine; HWDGE fixed cost => few, big chunks) ----
            nc.sync.dma_start(in_t[:, p_off:vu_off], in_d[:, p_off:vu_off])
            for g in range(HG):
                lo, hi = vu_off + g * g_c, vu_off + (g + 1) * g_c
                nc.sync.dma_start(in_t[:, lo:hi], in_d[:, lo:hi])
            wg_c = 4 * 384
            for g in range(HG):
                nc.sync.dma_start(
                    in_t[:, w_off + g * wg_c:w_off + (g + 1) * wg_c],
                    in_d[:, w_off + g * wg_c:w_off + (g + 1) * wg_c])
            if nb1:
                nc.sync.dma_start(in_t[:, u1_off:w_off],
                                  in_d[:, u1_off:w_off])

            # ---- views ----
            def vsl(h, lo, hi):   # V block [sig', n2] as lhsT
                g, i = divmod(h, 4)
                base = vu_off + g * g_c + i * 256
                return in_t[:, base + lo:base + hi]

            def usl(h, sb, jh):   # u cols (jl, b) for (h, sb, half)
                if jh == 0:
                    g, i = divmod(h, 4)
                    base = vu_off + g * g_c + vg_c + (i * 2 + sb) * C0
                    return in_t[:, base:base + C0]
                base = u1_off + (h * 2 + sb) * C1
                return in_t[:, base:base + C1]

            def wsl(h, lo, hi):   # [T1 | W1 | W2] block as lhsT
                base = w_off + h * 384
                return in_t[:, base + lo:base + hi]

            # broadcast compact p (pre|pim [128, HC]) to (h, b16) tiles
            pre_c = in_t[:, p_off:p_off + HC]
            pim_c = in_t[:, p_off + HC:vu_off]
            nc.gpsimd.tensor_copy(
                pre_t[:].rearrange("p (h b) -> p h b", h=HC),
                pre_c.rearrange("p (h o) -> p h o", o=1).broadcast_to(
                    (128, HC, 16)))
            nc.gpsimd.tensor_copy(
                pim_t[:].rearrange("p (h b) -> p h b", h=HC),
                pim_c.rearrange("p (h o) -> p h o", o=1).broadcast_to(
                    (128, HC, 16)))
            pre_v = pre_t[:].rearrange("p (h b) -> p h b", h=HC)
            pim_v = pim_t[:].rearrange("p (h b) -> p h b", h=HC)

            a0_4 = a0[:].rearrange("p (h jl b) -> p h jl b", h=HC, jl=8)
            a1_4 = (a1[:].rearrange("p (h jl b) -> p h jl b", h=HC, jl=8)
                    if nb1 else None)
            sy0_4 = sy0[:].rearrange("p (h jl b) -> p h jl b", h=HC, jl=8)
            sy1_4 = (sy1[:].rearrange("p (h jl b) -> p h jl b", h=HC, jl=8)
                     if nb1 else None)

            def aslot(j, nb):   # A_j as [p, h, b<=nb]
                jh, jl = divmod(j, 8)
                a4 = a0_4 if jh == 0 else a1_4
                return a4[:, :, jl, 0:nb]

            def eslot(c, nb):   # E_c (state entering chunk c)
                jh, jl = divmod(c, 8)
                sy4 = sy0_4 if jh == 0 else sy1_4
                return sy4[:, :, jl, 0:nb]

            # zero state feeding chunk 0 (rest of sy* dead lanes are
            # host-masked garbage)
            nc.gpsimd.memset(sy0_4[:, :, 0, :], 0.0)

            # ---- phase A: A_j = V^T u_j and P0_j = V_hat^T u0_j ----
            # (psum per 4h); jh1 emits h-groups 4-7 first (phase C jh0
            # ordering consumes 0-3 later)
            for jh in range(JH):
                if jh == 1 and not nb1:
                    break
                C = C0 if jh == 0 else C1
                a_dst = a0 if jh == 0 else a1
                p_dst = p0_0 if jh == 0 else p0_1
                order = ([0, 1, 2, 3, 4, 5, 6, 7] if jh == 0
                         else [4, 5, 6, 7, 0, 1, 2, 3])
                for hq in order:
                    a_ps = psum.tile([128, 4 * C], F32, name="a_ps",
                                     tag="ps")
                    p_ps = psum.tile([128, 4 * C], F32, name="p_ps",
                                     tag="ps")
                    for i in range(4):
                        h = 4 * hq + i
                        nc.tensor.matmul(a_ps[:, i * C:(i + 1) * C],
                                         vsl(h, 0, 128), usl(h, 0, jh),
                                         start=True, stop=False,
                                         skip_group_check=(i > 0))
                        nc.tensor.matmul(a_ps[:, i * C:(i + 1) * C],
                                         vsl(h, 128, 256), usl(h, 1, jh),
                                         start=False, stop=True,
                                         skip_group_check=(i > 0))
                        nc.tensor.matmul(p_ps[:, i * C:(i + 1) * C],
                                         vsl(h, 128, 256), usl(h, 0, jh),
                                         start=True, stop=True,
                                         skip_group_check=(i > 0))
                    dst = a_dst[:, 4 * hq * C:(4 * hq + 4) * C]
                    pdst = p_dst[:, 4 * hq * C:(4 * hq + 4) * C]
                    # A copies gate the scan: keep on DVE (fast) for jh0.
                    # P0 copies only gate phase C: Act (GPSIMD can't read
                    # PSUM).
                    if jh == 0:
                        nc.vector.tensor_copy(dst, a_ps[:])
                    else:
                        nc.scalar.copy(dst, a_ps[:])
                    nc.scalar.copy(pdst, p_ps[:])

            # ---- phase B: complex scan E_{j+1} = p*E_j + A_j ----
            # single full-h chain, writes directly into sy slots
            if nb_scan[0]:
                nc.gpsimd.tensor_copy(eslot(1, nb_scan[0]),
                                      aslot(0, nb_scan[0]))
            for j in range(1, 15):
                nb = nb_scan[j]
                if nb == 0:
                    break
                s_in = aslot(0, nb) if j == 1 else eslot(j, nb)
                m_a = spool.tile([128, HC * 16], BF16, name="m_a")
                swp = spool.tile([128, HC * 16], BF16, name="swp")
                m_b = spool.tile([128, HC * 16], BF16, name="m_b")
                tt = spool.tile([128, HC * 16], BF16, name="tt")
                m_a4 = m_a[:].rearrange("p (h b) -> p h b", h=HC)[:, :, 0:nb]
                swp4 = swp[:].rearrange("p (h b) -> p h b", h=HC)[:, :, 0:nb]
                m_b4 = m_b[:].rearrange("p (h b) -> p h b", h=HC)[:, :, 0:nb]
                tt4 = tt[:].rearrange("p (h b) -> p h b", h=HC)[:, :, 0:nb]
                pre4 = pre_v[:, :, 0:nb]
                pim4 = pim_v[:, :, 0:nb]
                nc.vector.tensor_copy(swp4[0:64], s_in[64:128])
                nc.vector.tensor_copy(swp4[64:128], s_in[0:64])
                nc.vector.tensor_mul(m_a4, pre4, s_in)
                nc.vector.tensor_mul(m_b4, pim4, swp4)
                nc.vector.tensor_add(tt4, m_a4, m_b4)
                nc.vector.tensor_add(eslot(j + 1, nb), tt4, aslot(j, nb))

            # ---- phase C: y = T1^T u + W1^T [P0|E] + W2^T E ----
            for jh in range(JH):
                if jh == 1 and not nb1:
                    break
                R = R0 if jh == 0 else R1
                y_d = y0_d if jh == 0 else y1_d
                sy_f = sy0 if jh == 0 else sy1
                p0_f = p0_0 if jh == 0 else p0_1
                for hg in range(HG):
                    y_sb = ypool.tile([128, 4 * 2 * R], BF16, name="y_sb")
                    for hp in range(2):
                        ps = psum.tile([128, 2 * 2 * R], F32, name="y_ps",
                                       tag="ps")
                        for i in range(2):
                            h = hg * 4 + hp * 2 + i
                            s_rhs = sy_f[:, h * R:(h + 1) * R]
                            p_rhs = p0_f[:, h * R:(h + 1) * R]
                            o = i * 2 * R
                            # tau half A (tau' 0..127)
                            nc.tensor.matmul(ps[:, o:o + R], wsl(h, 0, 128),
                                             usl(h, 0, jh),
                                             start=True, stop=False,
                                             skip_group_check=(i == 1))
                            nc.tensor.matmul(ps[:, o:o + R],
                                             wsl(h, 128, 256), s_rhs,
                                             start=False, stop=True,
                                             skip_group_check=(i == 1))
                            # tau half B (tau' 128..255)
                            nc.tensor.matmul(ps[:, o + R:o + 2 * R],
                                             wsl(h, 128, 256), p_rhs,
                                             start=True, stop=False,
                                             skip_group_check=True)
                            nc.tensor.matmul(ps[:, o + R:o + 2 * R],
                                             wsl(h, 0, 128), usl(h, 1, jh),
                                             start=False, stop=False,
                                             skip_group_check=True)
                            nc.tensor.matmul(ps[:, o + R:o + 2 * R],
                                             wsl(h, 256, 384), s_rhs,
                                             start=False, stop=True,
                                             skip_group_check=True)
                        # y copies: Act mostly, DVE for the jh1 tail
                        if jh == 1 and hg % 2 == 1:
                            nc.vector.tensor_copy(
                                y_sb[:, hp * 4 * R:(hp + 1) * 4 * R], ps[:])
                        else:
                            nc.scalar.copy(
                                y_sb[:, hp * 4 * R:(hp + 1) * 4 * R], ps[:])
                    nc.sync.dma_start(
                        y_d[:, hg * 8 * R:(hg + 1) * 8 * R], y_sb[:])

    nc.compile()
    return nc


_CACHE = {}


def _get_program(k_b):
    key = tuple(k_b)
    if key not in _CACHE:
        _CACHE[key] = _build_program(k_b)
    return _CACHE[key]


def _host_precompute(log_dt, C, log_A_real, A_imag, D):
    """Per-h weight blocks (fp64 internally)."""
    dt = np.exp(log_dt.astype(np.float64))
    A = -np.exp(log_A_real.astype(np.float64)) + 1j * A_imag.astype(np.float64)
    dtA = A * dt[:, None]
    w = np.exp(dtA)                                   # (H,N)
    Cc = C[..., 0].astype(np.float64) + 1j * C[..., 1].astype(np.float64)
    Cs = Cc * (np.exp(dtA) - 1.0) / A                 # (H,N)

    l = np.arange(T, dtype=np.float64)
    K = 2.0 * np.einsum('hn,hnl->hl', Cs, np.exp(dtA[:, :, None] * l)).real
    K[:, 0] += D.astype(np.float64)

    sig = np.arange(T)
    Vc = w[:, None, :] ** (T - sig)[None, :, None]    # (H,T,N)
    V_real = np.concatenate([Vc.real, Vc.imag], axis=2)  # (H,T,2N)

    tau = np.arange(T)
    Wc = Cs[:, :, None] * w[:, :, None] ** tau        # (H,N,T)
    W_real = np.concatenate([2 * Wc.real, -2 * Wc.imag], axis=1)  # (H,2N,T)

    p = w ** T                                        # (H,N)

    sp = np.arange(128)
    d = tau[None, :128] - sp[:, None]                 # (128, 128)
    T1 = np.where((d >= 0)[None], K[:, np.clip(d, 0, T - 1)], 0.0)
    return T1, V_real, W_real, p


def kernel(u, length, log_dt, C, log_A_real, A_imag, D, **_unused):
    u = np.asarray(u, dtype=np.float32)
    length = np.asarray(length).astype(np.int64)
    mask = (np.arange(L)[None, :] < length[:, None])
    u_m = (u * mask[:, None, :]).astype(np.float32)

    perm = np.argsort(-length, kind="stable")
    k_b = [int(min(J, (int(length[b]) + T - 1) // T)) for b in perm]
    nb_scan, nb_half = _plan(k_b)
    nb0, nb1 = nb_half
    C0, C1 = 8 * nb0, 8 * nb1
    R0, R1 = C0, C1
    HG = 8
    vg_c, u0g_c = 4 * 256, 4 * 2 * C0
    g_c = vg_c + u0g_c
    p_off = 0
    vu_off = 2 * HC
    u1_off = vu_off + 8 * g_c
    w_off = u1_off + HC * 2 * C1
    ICOLS = w_off + HC * 384

    T1, V_real, W_real, p = _host_precompute(
        np.asarray(log_dt), np.asarray(C), np.asarray(log_A_real),
        np.asarray(A_imag), np.asarray(D))

    # per-h weight tiles
    vwts = np.empty((H, 128, 256), dtype=np.float64)
    vwts[:, :, 0:128] = V_real[:, 0:128, :]      # lhsT [sig', n2]
    vwts[:, :, 128:256] = V_real[:, 128:256, :]
    vwts = vwts.astype(NP_BF16)
    wts = np.empty((H, 128, 384), dtype=np.float64)
    wts[:, :, 0:128] = T1                        # lhsT [sig', tau]
    wts[:, :, 128:384] = W_real                  # lhsT [n2, tau]
    wts = wts.astype(NP_BF16)

    # u packed layout per core: see _build_program
    u_s = u_m[perm]                                   # (B, H, L) sorted
    # (b, h, c16, sb2, sig128) -> chunk c = jh*8 + jl
    u_r = u_s.reshape(B, H, J, 2, 128)

    nc = _get_program(k_b)
    in_maps = []
    for c in range(NCORES):
        hs = slice(c * HC, (c + 1) * HC)
        buf = np.zeros((128, ICOLS), dtype=NP_BF16)
        vh = vwts[hs]                                 # (32, 128, 256)
        wh = wts[hs]                                  # (32, 128, 384)
        # chunk g: [v_g | u0_g]
        for g in range(HG):
            base = vu_off + g * g_c
            for i in range(4):
                h = g * 4 + i
                buf[:, base + i * 256:base + (i + 1) * 256] = vh[h]
            ub = base + vg_c
            for i in range(4):
                h = g * 4 + i
                for sb in range(2):
                    off = ub + (i * 2 + sb) * C0
                    # cols (jl, b) for jh0: chunk jl, batches 0:nb0
                    blk = u_r[0:nb0, c * HC + h, 0:8, sb, :]  # (b, jl, sig)
                    buf[:, off:off + C0] = blk.transpose(2, 1, 0).reshape(
                        128, 8 * nb0).astype(NP_BF16)
        # p chunk (compact): pre|pim [128, HC] each
        ph = p[hs]                                    # (32, 64) complex
        pr = np.empty((128, HC), dtype=np.float64)
        pi = np.empty((128, HC), dtype=np.float64)
        pr[0:64] = ph.real.T
        pr[64:128] = ph.real.T
        pi[0:64] = -ph.imag.T
        pi[64:128] = ph.imag.T
        buf[:, p_off:p_off + HC] = pr.astype(NP_BF16)
        buf[:, p_off + HC:vu_off] = pi.astype(NP_BF16)
        # u1 chunk
        if nb1:
            for h in range(HC):
                for sb in range(2):
                    off = u1_off + (h * 2 + sb) * C1
                    blk = u_r[0:nb1, c * HC + h, 8:16, sb, :]  # (b, jl, sig)
                    buf[:, off:off + C1] = blk.transpose(2, 1, 0).reshape(
                        128, 8 * nb1).astype(NP_BF16)
        # w chunks
        for h in range(HC):
            buf[:, w_off + h * 384:w_off + (h + 1) * 384] = wh[h]
        in_maps.append({"inp": np.ascontiguousarray(buf)})

    res = run_bass_kernel_spmd(nc, in_maps, core_ids=list(range(NCORES)))

    y = np.zeros((B, H, L), dtype=np.float32)
    for c in range(NCORES):
        hs = c * HC
        y0 = res.results[c]["y0"].astype(np.float32)  # [128, HC*2*R0]
        y0 = y0.reshape(128, HC, 2, 8, nb0)           # tau', h, half, jl, b
        # l = jl*256 + half*128 + tau'
        y0 = y0.transpose(4, 1, 3, 2, 0).reshape(nb0, HC, 8 * 256)
        y[0:nb0, hs:hs + HC, 0:2048] = y0
        if nb1:
            y1 = res.results[c]["y1"].astype(np.float32)
            y1 = y1.reshape(128, HC, 2, 8, nb1)
            y1 = y1.transpose(4, 1, 3, 2, 0).reshape(nb1, HC, 8 * 256)
            y[0:nb1, hs:hs + HC, 2048:4096] = y1
    inv = np.empty(B, dtype=np.int64)
    inv[perm] = np.arange(B)
    y = y[inv]
    y = np.where(mask[:, None, :], y, np.float32(0.0))
    return np.ascontiguousarray(y)
